# revision 11
# baseline (speedup 1.0000x reference)
"""Multi-head attention (B=2, S=2048, E=1024, H=16) on 8 Trainium2 NeuronCores.

Sharding: core c handles batch b=c//4 and head group g=c%4 (4 heads each).
hidden_states[b] is replicated to the 4 cores of batch b (pre-transposed and
cast to bf16 on host so the contraction dim E lands on SBUF partitions with
plain contiguous DMAs). Each core computes q/k/v projections for its heads,
transposed-layout attention (scores^T = k q'^T so softmax reduces over the
PSUM partition dim via a ones-matmul), and a partial output projection over
its 256 E-dims. The host sums the 4 partials per batch and adds bo.

Bias handling: softmax over t is invariant to per-query constants, so the
k-bias drops out entirely and the q-bias is folded into q' = q + bq. The
v-bias is a post-softmax additive constant (softmax rows sum to 1), applied
after normalization. bo is added on host.
"""

import sys

if "/opt/trn_rl_repo" not in sys.path:
    sys.path.insert(0, "/opt/trn_rl_repo")

import numpy as np
import ml_dtypes

import concourse.bass as bass
import concourse.tile as tile
from concourse import mybir
from concourse.bass_utils import run_bass_kernel_spmd
from concourse.vector_clock import ScopedClock

B, S, E, H = 2, 2048, 1024, 16
DH = E // H  # 64
N_CORES = 8
HEADS_PER_CORE = 4  # 2 pairs
EL = HEADS_PER_CORE * DH  # 256 local E-dims per core

F32 = mybir.dt.float32
BF16 = mybir.dt.bfloat16
BF16_NP = ml_dtypes.bfloat16

ST = 512  # s_tile width (softmax free dim per psum bank)
N_ST = S // ST  # 4
N_TC = S // 128  # 16 t-chunks
N_EC = E // 128  # 8 e-chunks


def _patch_tail_drain():
    """walrus CoreV3 setupSyncWait allows only 1 sem wait on an SP Drain; Tile's
    kernel-tail drain carries one wait per live processor. Split the waits
    across consecutive drains (mutating via nc.inst_map, whose objects are what
    to_json_bytes serializes)."""
    if getattr(tile.TileContext, "_drain_patched", False):
        return

    def _drain_and_barrier(self, tick_clock, wait_clock):
        nc = self.nc
        drain_inst = nc.sync.drain()
        wait_clock.add_sem_waits(
            drain_inst.ins, ScopedClock({None: tick_clock.global_clock})
        )
        inst = nc.inst_map[drain_inst.ins.name]
        w = list(inst.sync_info.on_wait) if inst.sync_info else []
        if len(w) > 1:
            si = inst.sync_info
            si.on_wait = w[:1]
            inst.sync_info = si
            for i in range(1, len(w)):
                d2 = nc.sync.drain()
                i2 = nc.inst_map[d2.ins.name]
                si2 = i2.sync_info or mybir.SyncInfo(on_wait=[], on_update=[])
                si2.on_wait = [w[i]]
                i2.sync_info = si2
        nc.all_engine_barrier()
        assert self.sems is not None
        popped = nc._tile_sem_poison_stack.pop()
        assert popped is self._sem_poison
        nc.clear_and_free_semaphores(list(self.sems.allocated().values()))
        nc.all_engine_barrier()

    tile.TileContext._drain_and_barrier = _drain_and_barrier
    tile.TileContext._drain_patched = True


def _split_multi_waits(nc):
    """The walrus build in this environment accepts only ONE sem-wait command
    per instruction, but Tile's wait-assignment attaches several. Hoist excess
    waits onto dedicated same-engine no-op carrier instructions inserted
    immediately before the owner (same engine-stream position, identical
    semantics)."""
    f = nc.m.functions[0]
    blocks = list(f.blocks)
    carriers: dict[str, list] = {}
    created = set()
    for blk in blocks:
        for inst in blk.instructions:
            if inst.sync_info and len(inst.sync_info.on_wait) > 1:
                w = list(inst.sync_info.on_wait)
                cs = []
                for wx in w[:-1]:
                    # engine nop() appends to nc.cur_bb; it is re-homed below
                    nop = nc.engines[inst.engine].nop(nofuse=True).ins
                    nop.sync_info = mybir.SyncInfo(on_wait=[wx], on_update=[])
                    cs.append(nop)
                    created.add(nop.name)
                si = inst.sync_info
                si.on_wait = [w[-1]]
                inst.sync_info = si
                carriers[inst.name] = cs
    if not carriers:
        return
    for blk in blocks:
        rebuilt = []
        for i in blk.instructions:
            if i.name in created:
                continue
            rebuilt.extend(carriers.get(i.name, ()))
            rebuilt.append(i)
        blk.instructions = rebuilt


def build_bass():
    """Build the per-core Bass program (identical on all 8 cores)."""
    _patch_tail_drain()
    nc = bass.Bass("TRN2", target_bir_lowering=False, debug=False)

    xt_d = nc.dram_tensor("xt", [E, S], BF16, kind="ExternalInput").ap()
    wq_d = nc.dram_tensor("wq", [E, EL], BF16, kind="ExternalInput").ap()
    wk_d = nc.dram_tensor("wk", [E, EL], BF16, kind="ExternalInput").ap()
    wv_d = nc.dram_tensor("wv", [E, EL], BF16, kind="ExternalInput").ap()
    wo_d = nc.dram_tensor("wo", [EL, E], BF16, kind="ExternalInput").ap()
    bq_d = nc.dram_tensor("bq2", [128, 2], F32, kind="ExternalInput").ap()
    bv_d = nc.dram_tensor("bv2", [128, 2], F32, kind="ExternalInput").ap()
    out_d = nc.dram_tensor("out", [S, E], F32, kind="ExternalOutput").ap()

    EXP = mybir.ActivationFunctionType.Exp
    ADD = mybir.AluOpType.add
    MULT = mybir.AluOpType.mult

    with tile.TileContext(nc) as tc:
        with (
            tc.tile_pool(name="const", bufs=1) as const_pool,
            tc.tile_pool(name="xw", bufs=1) as xw_pool,
            tc.tile_pool(name="qkv", bufs=1) as qkv_pool,
            tc.tile_pool(name="exps", bufs=3) as exp_pool,
            tc.tile_pool(name="ctxn", bufs=4) as ctxn_pool,
            tc.tile_pool(name="small", bufs=4) as small_pool,
            tc.tile_pool(name="rb", bufs=4) as rb_pool,
            tc.tile_pool(name="outs", bufs=3) as out_pool,
            tc.tile_pool(name="pp", bufs=2, space="PSUM") as pp_ps,
            tc.tile_pool(name="sc", bufs=2, space="PSUM") as sc_ps,
            tc.tile_pool(name="ctx", bufs=1, space="PSUM") as ctx_ps_pool,
            tc.tile_pool(name="den", bufs=1, space="PSUM") as den_ps_pool,
        ):
            # ---- constants and weights
            ones_sb = const_pool.tile([128, 1], BF16)
            nc.vector.memset(ones_sb[:], 1.0)
            ones1_sb = const_pool.tile([1, 64], mybir.dt.float16)
            nc.vector.memset(ones1_sb[:], 1.0)
            bq_sb = const_pool.tile([128, 2], F32)
            nc.sync.dma_start(bq_sb[:], bq_d[:])
            bv_sb = const_pool.tile([128, 2], F32)
            nc.sync.dma_start(bv_sb[:], bv_d[:])

            wq_sb = xw_pool.tile([128, N_EC, EL], BF16)
            nc.sync.dma_start(wq_sb[:], wq_d.rearrange("(o p) d -> p o d", p=128))
            wk_sb = xw_pool.tile([128, N_EC, EL], BF16)
            nc.sync.dma_start(wk_sb[:], wk_d.rearrange("(o p) d -> p o d", p=128))
            wv_sb = xw_pool.tile([128, N_EC, EL], BF16)
            nc.sync.dma_start(wv_sb[:], wv_d.rearrange("(o p) d -> p o d", p=128))
            wo_sb = xw_pool.tile([128, 2, E], BF16)
            nc.sync.dma_start(wo_sb[:], wo_d.rearrange("(o p) n -> p o n", p=128))

            xt_sb = xw_pool.tile([128, N_EC, S], BF16)
            for ec in range(N_EC):
                nc.sync.dma_start(xt_sb[:, ec, :], xt_d[128 * ec : 128 * (ec + 1), :])

            # ---- projections: q'^T (with bias), k^T, v (natural layout)
            qT = [qkv_pool.tile([128, S], BF16, name=f"qT{p}") for p in range(2)]
            kT = [qkv_pool.tile([128, S], BF16, name=f"kT{p}") for p in range(2)]
            v_sb = qkv_pool.tile([128, N_TC, EL], BF16)

            for p in range(2):
                dlo, dhi = 128 * p, 128 * (p + 1)
                for st in range(N_ST):
                    slo, shi = ST * st, ST * (st + 1)
                    ps_q = pp_ps.tile([128, ST], F32, tag="pp")
                    for ec in range(N_EC):
                        nc.tensor.matmul(
                            ps_q[:],
                            wq_sb[:, ec, dlo:dhi],
                            xt_sb[:, ec, slo:shi],
                            start=(ec == 0),
                            stop=(ec == N_EC - 1),
                        )
                    nc.vector.tensor_scalar(
                        qT[p][:, slo:shi], ps_q[:], bq_sb[:, p : p + 1], None, ADD
                    )
                    ps_k = pp_ps.tile([128, ST], F32, tag="pp")
                    for ec in range(N_EC):
                        nc.tensor.matmul(
                            ps_k[:],
                            wk_sb[:, ec, dlo:dhi],
                            xt_sb[:, ec, slo:shi],
                            start=(ec == 0),
                            stop=(ec == N_EC - 1),
                        )
                    nc.vector.tensor_copy(kT[p][:, slo:shi], ps_k[:])

            for tt in range(N_TC):
                ps_v = pp_ps.tile([128, ST], F32, tag="pp")
                for ec in range(N_EC):
                    nc.tensor.matmul(
                        ps_v[:, :EL],
                        xt_sb[:, ec, 128 * tt : 128 * (tt + 1)],
                        wv_sb[:, ec, :],
                        start=(ec == 0),
                        stop=(ec == N_EC - 1),
                    )
                nc.vector.tensor_copy(v_sb[:, tt, :], ps_v[:, :EL])

            # ---- attention + output projection, per s_tile
            for st in range(N_ST):
                slo, shi = ST * st, ST * (st + 1)
                cns = []
                for p in range(2):
                    ctx_ps = ctx_ps_pool.tile([128, ST], F32)
                    den_ps = den_ps_pool.tile([128, ST], F32)
                    for tc in range(N_TC):
                        tlo, thi = 128 * tc, 128 * (tc + 1)
                        sc = sc_ps.tile([128, 2 * ST], F32)
                        nc.tensor.matmul(
                            sc[:, :ST],
                            kT[p][0:64, tlo:thi],
                            qT[p][0:64, slo:shi],
                            start=True,
                            stop=True,
                        )
                        nc.tensor.matmul(
                            sc[:, ST:],
                            kT[p][64:128, tlo:thi],
                            qT[p][64:128, slo:shi],
                            start=True,
                            stop=True,
                        )
                        ex = exp_pool.tile([128, 2 * ST], BF16)
                        nc.scalar.activation(ex[:], sc[:], EXP, scale=0.125)
                        first, last = tc == 0, tc == N_TC - 1
                        nc.tensor.matmul(
                            ctx_ps[0:64, :],
                            v_sb[:, tc, 128 * p : 128 * p + 64],
                            ex[:, :ST],
                            start=first,
                            stop=last,
                        )
                        nc.tensor.matmul(
                            ctx_ps[64:128, :],
                            v_sb[:, tc, 128 * p + 64 : 128 * (p + 1)],
                            ex[:, ST:],
                            start=first,
                            stop=last,
                        )
                        nc.tensor.matmul(
                            den_ps[0:1, :], ones_sb[:], ex[:, :ST],
                            start=first, stop=last,
                        )
                        nc.tensor.matmul(
                            den_ps[64:65, :], ones_sb[:], ex[:, ST:],
                            start=first, stop=last,
                        )
                    # normalize: ctx / denom + bv  (denom recip rows broadcast
                    # across the 64 head partitions via a stride-0 DMA read)
                    r0 = small_pool.tile([1, ST], mybir.dt.float16, tag="r0")
                    r1 = small_pool.tile([1, ST], mybir.dt.float16, tag="r1")
                    with nc.allow_low_precision(
                        reason="fp16 reciprocal rows: 5e-4 rel err, well under bf16 ctx"
                    ):
                        nc.vector.reciprocal(r0[:], den_ps[0:1, :])
                        nc.vector.reciprocal(r1[:], den_ps[64:65, :])
                    rbp = pp_ps.tile([128, ST], F32, tag="pp")
                    nc.tensor.matmul(
                        rbp[0:64, :],
                        ones1_sb[:],
                        r0[:],
                        start=True,
                        stop=True,
                    )
                    nc.tensor.matmul(
                        rbp[64:128, :],
                        ones1_sb[:],
                        r1[:],
                        start=True,
                        stop=True,
                    )
                    rb = rb_pool.tile([128, ST], F32, tag="rb")
                    nc.vector.tensor_copy(rb[:], rbp[:])
                    cn = ctxn_pool.tile([128, ST], BF16)
                    nc.vector.tensor_tensor(
                        cn[0:64, :], ctx_ps[0:64, :], rb[0:64, :], MULT
                    )
                    nc.vector.tensor_tensor(
                        cn[64:128, :], ctx_ps[64:128, :], rb[64:128, :], MULT
                    )
                    nc.vector.tensor_scalar(
                        cn[:], cn[:], bv_sb[:, p : p + 1], None, ADD
                    )
                    cns.append(cn)
                # output projection for this s_tile
                for ss in range(ST // 128):
                    srow = slo + 128 * ss
                    for nt in range(E // ST):
                        ps_o = pp_ps.tile([128, ST], F32, tag="pp")
                        nc.tensor.matmul(
                            ps_o[:],
                            cns[0][:, 128 * ss : 128 * (ss + 1)],
                            wo_sb[:, 0, ST * nt : ST * (nt + 1)],
                            start=True,
                            stop=False,
                        )
                        nc.tensor.matmul(
                            ps_o[:],
                            cns[1][:, 128 * ss : 128 * (ss + 1)],
                            wo_sb[:, 1, ST * nt : ST * (nt + 1)],
                            start=False,
                            stop=True,
                        )
                        ob = out_pool.tile([128, ST], F32)
                        nc.vector.tensor_copy(ob[:], ps_o[:])
                        nc.sync.dma_start(
                            out_d[srow : srow + 128, ST * nt : ST * (nt + 1)], ob[:]
                        )
    _split_multi_waits(nc)
    return nc


_NC = None


def _get_nc():
    global _NC
    if _NC is None:
        _NC = build_bass()
    return _NC


def make_in_maps(hidden_states, Wq, bq, Wk, bk, Wv, bv, Wo):
    """Host-side sharding/layout prep. Returns list of 8 per-core input dicts."""
    hs = np.asarray(hidden_states, dtype=np.float32)
    Wq = np.asarray(Wq, dtype=np.float32)
    Wk = np.asarray(Wk, dtype=np.float32)
    Wv = np.asarray(Wv, dtype=np.float32)
    Wo = np.asarray(Wo, dtype=np.float32)
    bq = np.asarray(bq, dtype=np.float32)
    bv = np.asarray(bv, dtype=np.float32)

    xt = [
        np.ascontiguousarray(hs[b].T).astype(BF16_NP) for b in range(B)
    ]  # [E, S] bf16
    in_maps = []
    for c in range(N_CORES):
        b, g = divmod(c, N_CORES // B)
        h0 = HEADS_PER_CORE * g
        hsl = slice(h0, h0 + HEADS_PER_CORE)
        # [H_loc, E, DH] -> [E, H_loc*DH] head-major columns
        wq_c = np.ascontiguousarray(
            Wq[hsl].transpose(1, 0, 2).reshape(E, EL)
        ).astype(BF16_NP)
        wk_c = np.ascontiguousarray(
            Wk[hsl].transpose(1, 0, 2).reshape(E, EL)
        ).astype(BF16_NP)
        wv_c = np.ascontiguousarray(
            Wv[hsl].transpose(1, 0, 2).reshape(E, EL)
        ).astype(BF16_NP)
        wo_c = np.ascontiguousarray(Wo[EL * g : EL * (g + 1), :]).astype(BF16_NP)
        bq_c = np.ascontiguousarray(bq[hsl].reshape(EL).reshape(2, 128).T)
        bv_c = np.ascontiguousarray(bv[hsl].reshape(EL).reshape(2, 128).T)
        in_maps.append(
            {
                "xt": xt[b],
                "wq": wq_c,
                "wk": wk_c,
                "wv": wv_c,
                "wo": wo_c,
                "bq2": bq_c,
                "bv2": bv_c,
            }
        )
    return in_maps


def kernel(hidden_states, mask, Wq, bq, Wk, bk, Wv, bv, Wo, bo, **run_kwargs):
    """Full-input entry point. mask is all-ones per the problem spec (ignored)."""
    nc = _get_nc()
    in_maps = make_in_maps(hidden_states, Wq, bq, Wk, bk, Wv, bv, Wo)
    res = run_bass_kernel_spmd(nc, in_maps, core_ids=list(range(N_CORES)), **run_kwargs)
    bo = np.asarray(bo, dtype=np.float32)
    out = np.zeros((B, S, E), dtype=np.float32)
    for c in range(N_CORES):
        out[c // (N_CORES // B)] += res.results[c]["out"]
    out += bo
    kernel.last_results = res
    return out
